# revision 10
# baseline (speedup 1.0000x reference)
"""DiffusionLoss Trainium2 kernel: 8-core SPMD Bass/Tile implementation.

Math: heat(tau) = expm(-tau * (I - W)) = e^{-tau} * exp(tau * W), where
W = D^{-1/2} A D^{-1/2} is the normalized adjacency (symmetric, ||W||_2 <= 1).
heat(5) = sum_k e^{-5} 5^k/k! W^k is evaluated with a degree-15 polynomial via
Paterson-Stockmeyer (chunk 4) and heat(10) = heat(5)^2.

Parallelization: column-block 1D sharding. Core c owns columns
[512c, 512c+512). Every big matmul is (symmetric full matrix) @ (local
column block); the full matrix serves as the pre-transposed stationary
operand. Full A is built redundantly on every core from the replicated
positions; W^4 and heat(5) are assembled with two AllGathers. Per-column
sums / sums of squares are computed on device; the final CV reduction runs
on the host in float64.

V1 = W E_blk needs no matmul phase: pass A recomputes the block columns of
A directly from a per-core augRb input (contraction 5), and V1/T1 are
elementwise scalings of that block. Powers V2..V4 stream the raw adjacency
as lhsT with the D-scalings folded into PSUM evictions (T_p = D^2 A T_{p-1}).
Big matmul phases: V2, V3, V4, Horner x3, H10 = 7 (vs 10 in the deg-24
chunk-5 predecessor).
"""

import math

import numpy as np
import ml_dtypes

import concourse.bass as bass
import concourse.mybir as mybir
import concourse.tile as tile
from concourse import bacc
from concourse.bass_utils import run_bass_kernel_spmd
from concourse.masks import make_identity

N = 4096
P = 128
NT = N // P  # 32 partition tiles
B = 512  # columns per core
NB = B // P  # 4
NCH = N // B  # 8 free-dim chunks
C = 8  # cores
TAU = 5.0
DEG = 15
CHK = 4  # PS chunk
NQ = (DEG + 1) // CHK  # 4 chunks
MAX_DISTANCE = 50.0

F32 = mybir.dt.float32
BF16 = mybir.dt.bfloat16
AF = mybir.ActivationFunctionType
OP = mybir.AluOpType

# c[k] = e^{-tau} tau^k / k!
COEF = [math.exp(-TAU) * TAU**k / math.factorial(k) for k in range(DEG + 1)]


def build_nc():
    nc = bacc.Bacc(
        "TRN2",
        target_bir_lowering=False,
        debug=False,
        enable_asserts=True,
        num_devices=C,
    )
    augL_in = nc.dram_tensor("augL", [5, N], BF16, kind="ExternalInput").ap()
    augR_in = nc.dram_tensor("augR", [5, N], BF16, kind="ExternalInput").ap()
    augRb_in = nc.dram_tensor("augRb", [5, B], BF16, kind="ExternalInput").ap()
    eye_blk = nc.dram_tensor("eye_blk", [N, B], BF16, kind="ExternalInput").ap()
    out = nc.dram_tensor("out", [4, B], F32, kind="ExternalOutput").ap()

    with tile.TileContext(nc) as tc:
        with (
            tc.tile_pool(name="sb", bufs=1) as sb,  # persistents
            tc.tile_pool(name="bigf", bufs=2) as bigf,  # [128, 4096] f32 tiles
            tc.tile_pool(name="ch", bufs=1) as chp,  # rotating smaller tiles
            tc.tile_pool(name="lt", bufs=2) as ltp,  # lhsT strips
            tc.tile_pool(name="ps", bufs=4, space="PSUM") as psp,
            tc.tile_pool(name="pstat", bufs=1, space="PSUM") as pstat,
            tc.tile_pool(name="dram", bufs=1, space="DRAM") as dram,
        ):
            # ---------------- persistents ----------------
            augLs = sb.tile([5, N], BF16, name="augLs")
            augRs = sb.tile([5, N], BF16, name="augRs")
            augRbs = sb.tile([5, B], BF16, name="augRbs")
            eye128 = sb.tile([P, P], F32, name="eye128")
            mask128 = sb.tile([P, P], F32, name="mask128")
            onesf = sb.tile([P, 1], F32, name="onesf")
            ones1 = sb.tile([1, P], F32, name="ones1")
            epsb = sb.tile([P, 1], F32, name="epsb")
            degraw = sb.tile([P, NT], F32, name="degraw")
            degcol = sb.tile([P, NT], F32, name="degcol")
            dsq = sb.tile([P, NT], F32, name="dsq")
            dinvcol = sb.tile([P, NT], F32, name="dinvcol")
            dinv2col = sb.tile([P, NT], F32, name="dinv2col")
            dinvb16 = sb.tile([P, NT], BF16, name="dinvb16")
            dcolb = sb.tile([P, B], F32, name="dcolb")  # d[block cols] bcast
            vbufA = sb.tile([P, NT, B], BF16, name="vbufA")
            vbufB = sb.tile([P, NT, B], BF16, name="vbufB")
            acc_cs5 = sb.tile([1, B], F32, name="acc_cs5")
            acc_ss5 = sb.tile([1, B], F32, name="acc_ss5")
            acc_cs10 = sb.tile([1, B], F32, name="acc_cs10")
            acc_ss10 = sb.tile([1, B], F32, name="acc_ss10")

            # ---------------- DRAM scratch ----------------
            adjd = dram.tile([N, N], BF16, name="adjd")
            vf = [dram.tile([N, B], BF16, name=f"vf{p}") for p in range(1, 4)]
            qd = [dram.tile([N, B], BF16, name=f"qd{j}") for j in range(NQ)]
            drowd = dram.tile([1, B], F32, name="drowd")
            SPL = 8
            HQ = N // SPL
            cc_in1 = [
                dram.tile([HQ, B], BF16, name=f"cc_in1{q}") for q in range(SPL)
            ]
            cc_w4 = [
                dram.tile([C * HQ, B], BF16, name=f"cc_w4{q}", addr_space="Shared")
                for q in range(SPL)
            ]
            cc_in2 = [
                dram.tile([HQ, B], BF16, name=f"cc_in2{q}") for q in range(SPL)
            ]
            cc_h5 = [
                dram.tile([C * HQ, B], BF16, name=f"cc_h5{q}", addr_space="Shared")
                for q in range(SPL)
            ]

            # tiled DRAM views
            adj_t = adjd.rearrange("(t p) n -> t p n", p=P)
            adj_strips = adjd.rearrange("(kc p) (mt c) -> mt p kc c", p=P, c=P)

            def split_strips(bufs_):
                return [
                    b.rearrange("(r kc p) (q c) -> r q p kc c", r=C, p=P, c=P)
                    for b in bufs_
                ]

            ccw4_s = split_strips(cc_w4)
            cch5_s = split_strips(cc_h5)
            eyeb_t = eye_blk.rearrange("(t p) n -> t p n", p=P)
            vf_t = [v.rearrange("(t p) n -> t p n", p=P) for v in vf]
            qd_t = [q.rearrange("(t p) n -> t p n", p=P) for q in qd]
            cc1_t = [b.rearrange("(t p) n -> t p n", p=P) for b in cc_in1]
            cc2_t = [b.rearrange("(t p) n -> t p n", p=P) for b in cc_in2]
            TQ = NT // SPL  # row-tiles per split

            # ---------------- setup ----------------
            nc.sync.dma_start(augLs[:], augL_in)
            nc.sync.dma_start(augRs[:], augR_in)
            nc.sync.dma_start(augRbs[:], augRb_in)
            make_identity(nc, eye128[:])
            nc.vector.tensor_scalar(
                mask128[:], eye128[:], -1.0, 1.0, op0=OP.mult, op1=OP.add
            )
            nc.vector.memset(onesf[:], 1.0)
            nc.vector.memset(ones1[:], 1.0)
            nc.vector.memset(epsb[:], 1e-6)
            nc.vector.memset(acc_cs5[:], 0.0)
            nc.vector.memset(acc_ss5[:], 0.0)
            nc.vector.memset(acc_cs10[:], 0.0)
            nc.vector.memset(acc_ss10[:], 0.0)

            # eye tiles in SBUF (bf16) for the d-row extraction matmuls
            eye_v = eye_blk.rearrange("(t p) n -> p t n", p=P)
            nc.sync.dma_start(vbufA[:], eye_v)

            # ---------------- pass A: adjacency + degree + A block ----------
            # d2[m, n] = augL[:, m] . augR[:, n] = |x_m|^2 + |x_n|^2 - 2 x_m.x_n
            for t in range(NT):
                big = bigf.tile([P, N], F32, tag="bigf")
                for nn in range(NCH):
                    d2ps = psp.tile([P, B], F32, tag="mm")
                    nc.tensor.matmul(
                        d2ps[:],
                        augLs[:, t * P : (t + 1) * P],
                        augRs[:, nn * B : (nn + 1) * B],
                        start=True,
                        stop=True,
                    )
                    nc.vector.tensor_scalar_max(
                        big[:, nn * B : (nn + 1) * B], d2ps[:], 0.0
                    )
                nc.scalar.activation(big[:], big[:], AF.Sqrt)
                nc.scalar.activation(
                    big[:],
                    big[:],
                    AF.Sigmoid,
                    scale=-1.0 / MAX_DISTANCE,
                    bias=1.0,
                    accum_out=degraw[:, t : t + 1],
                )
                # extract the (unmasked) diagonal, zero it, fix the degree
                dg = big[:, t * P : (t + 1) * P]
                dtmp = chp.tile([P, P], F32, tag="dtmp", bufs=2)
                nc.vector.tensor_tensor(dtmp[:], dg, eye128[:], op=OP.mult)
                diagv = chp.tile([P, 1], F32, tag="diagv", bufs=2)
                nc.vector.tensor_reduce(
                    diagv[:], dtmp[:], axis=mybir.AxisListType.X, op=OP.add
                )
                nc.vector.tensor_tensor(dg, dg, mask128[:], op=OP.mult)
                nc.vector.tensor_tensor(
                    degcol[:, t : t + 1], degraw[:, t : t + 1], diagv[:],
                    op=OP.subtract,
                )
                for h in range(2):
                    abf = chp.tile([P, N // 2], BF16, tag="b4k", bufs=2)
                    nc.vector.tensor_copy(
                        abf[:], big[:, h * (N // 2) : (h + 1) * (N // 2)]
                    )
                    nc.sync.dma_start(
                        adj_t[t][:, h * (N // 2) : (h + 1) * (N // 2)], abf[:]
                    )

            # ---------------- pass B: dinv = 1/sqrt(deg + 1e-6) --------------
            nc.scalar.activation(dsq[:], degcol[:], AF.Sqrt, bias=epsb[:])
            nc.vector.reciprocal(dinvcol[:], dsq[:])
            nc.vector.tensor_tensor(dinv2col[:], dinvcol[:], dinvcol[:], op=OP.mult)
            nc.vector.tensor_copy(dinvb16[:], dinvcol[:])

            # d over the block columns, broadcast to all partitions:
            # drow = sum_t dinv[:, t]^T @ eye_blk[t]  (selects block entries)
            drps = pstat.tile([1, B], F32, tag="statps", bufs=3)
            for t in range(NT):
                nc.tensor.matmul(
                    drps[:],
                    dinvb16[:, t : t + 1],
                    vbufA[:, t, :],
                    start=(t == 0),
                    stop=(t == NT - 1),
                )
            drsb = chp.tile([1, B], F32, tag="drsb")
            nc.vector.tensor_copy(drsb[:], drps[0:1, :])
            nc.sync.dma_start(drowd[:], drsb[:])
            drl = chp.tile([1, B], F32, tag="drl")
            nc.sync.dma_start(drl[:], drowd[:])
            dbps = pstat.tile([P, B], F32, tag="bcast", bufs=1)
            nc.tensor.matmul(dbps[:], ones1[:], drl[:], start=True, stop=True)
            nc.vector.tensor_copy(dcolb[:], dbps[:])

            # ---------------- V1 / T1 from recomputed A block ---------------
            # A_blk[t] from aug vectors (contraction 5), diag zeroed via
            # eye_blk; V1[t] = d_row * A_blk[t] * d_col -> vf[0];
            # T1[t] = d_row^2 * A_blk[t] * d_col -> vbufA (rhs for V2 phase).
            for t in range(NT):
                bps = psp.tile([P, B], F32, tag="mm")
                nc.tensor.matmul(
                    bps[:],
                    augLs[:, t * P : (t + 1) * P],
                    augRbs[:],
                    start=True,
                    stop=True,
                )
                blkf = chp.tile([P, B], F32, tag="blkf", bufs=2)
                nc.vector.tensor_scalar_max(blkf[:], bps[:], 0.0)
                nc.scalar.activation(blkf[:], blkf[:], AF.Sqrt)
                nc.scalar.activation(
                    blkf[:], blkf[:], AF.Sigmoid,
                    scale=-1.0 / MAX_DISTANCE, bias=1.0,
                )
                # zero the in-block diagonal: A_blk *= (1 - eye_blk[t])
                eyt = chp.tile([P, B], BF16, tag="eyt", bufs=3)
                nc.sync.dma_start(eyt[:], eyeb_t[t])
                emsk = chp.tile([P, B], F32, tag="emsk", bufs=2)
                nc.vector.tensor_scalar(
                    emsk[:], eyt[:], -1.0, 1.0, op0=OP.mult, op1=OP.add
                )
                nc.vector.tensor_tensor(blkf[:], blkf[:], emsk[:], op=OP.mult)
                v1f = chp.tile([P, B], F32, tag="v1f", bufs=2)
                nc.scalar.activation(
                    v1f[:], blkf[:], AF.Copy, scale=dinvcol[:, t : t + 1]
                )
                v1b = chp.tile([P, B], BF16, tag="evb", bufs=3)
                nc.vector.tensor_tensor(v1b[:], v1f[:], dcolb[:], op=OP.mult)
                nc.sync.dma_start(vf_t[0][t], v1b[:])
                t1f = chp.tile([P, B], F32, tag="t1f", bufs=2)
                nc.scalar.activation(
                    t1f[:], blkf[:], AF.Copy, scale=dinv2col[:, t : t + 1]
                )
                nc.vector.tensor_tensor(
                    vbufA[:, t, :], t1f[:], dcolb[:], op=OP.mult
                )

            # ---------------- big matmul helper ----------------
            def mm_phase(strips, rhs, evac, mid=None):
                for mt in range(NT):
                    lt = ltp.tile([P, NT, P], BF16, tag="lt")
                    src = strips(mt)
                    if isinstance(src, (tuple, list)):
                        npc = NT // len(src)
                        for qi, sq in enumerate(src):
                            nc.sync.dma_start(
                                lt[:, qi * npc : (qi + 1) * npc, :], sq
                            )
                    else:
                        nc.sync.dma_start(lt[:], src)
                    ps = psp.tile([P, B], F32, tag="mm")
                    for kc in range(NT):
                        nc.tensor.matmul(
                            ps[:],
                            lt[:, kc, :],
                            rhs[:, kc, :],
                            start=(kc == 0),
                            stop=(kc == NT - 1),
                        )
                    evac(mt, ps)
                    if mid is not None and mt in mid:
                        for fn in mid[mt]:
                            fn()

            def stat_pair(rf, cs_acc, ss_acc):
                csps = pstat.tile([1, B], F32, tag="statps", bufs=3)
                nc.tensor.matmul(csps[:], onesf[:], rf[:], start=True, stop=True)
                nc.vector.tensor_tensor(cs_acc[:], cs_acc[:], csps[0:1, :], op=OP.add)
                sqt = chp.tile([P, B], F32, tag="sqt", bufs=2)
                nc.vector.tensor_tensor(sqt[:], rf[:], rf[:], op=OP.mult)
                ssps = pstat.tile([1, B], F32, tag="statps", bufs=3)
                nc.tensor.matmul(ssps[:], onesf[:], sqt[:], start=True, stop=True)
                nc.vector.tensor_tensor(ss_acc[:], ss_acc[:], ssps[0:1, :], op=OP.add)

            # ---------------- powers V2..V4 ----------------
            # T_p := D W^p E ; PSUM = A @ T_{p-1} ; V_p = D PSUM ; T_p = D^2 PSUM
            bufs = [vbufA, vbufB]

            def evac_power(mt, ps, p, nxt):
                if p < CHK:
                    rf = chp.tile([P, B], BF16, tag="evb", bufs=3)
                    nc.scalar.activation(
                        rf[:], ps[:], AF.Copy, scale=dinvcol[:, mt : mt + 1]
                    )
                    nc.sync.dma_start(vf_t[p - 1][mt], rf[:])
                    nc.vector.tensor_scalar_mul(
                        nxt[:, mt, :], ps[:], dinv2col[:, mt : mt + 1]
                    )
                else:
                    vb = chp.tile([P, B], BF16, tag="evb", bufs=3)
                    nc.scalar.activation(
                        vb[:], ps[:], AF.Copy, scale=dinvcol[:, mt : mt + 1]
                    )
                    nc.sync.dma_start(cc1_t[mt // TQ][mt % TQ], vb[:])

            def gather(cin, cout):
                def run():
                    nc.gpsimd.collective_compute(
                        "AllGather",
                        OP.bypass,
                        replica_groups=[list(range(C))],
                        ins=[cin[:]],
                        outs=[cout[:]],
                    )

                return run

            def qpart_tile(t, js):
                # Q_j = c[4j] I + c[4j+1] V1 + c[4j+2] V2 + c[4j+3] V3 (bf16)
                eyt = chp.tile([P, B], BF16, tag="eyt", bufs=3)
                nc.sync.dma_start(eyt[:], eyeb_t[t])
                vts = chp.tile([P, 3, B], BF16, tag="vts", bufs=2)
                for r in range(3):
                    nc.sync.dma_start(vts[:, r, :], vf_t[r][t])
                for j in js:
                    qa = chp.tile([P, B], F32, tag="qa", bufs=3)
                    nc.vector.tensor_scalar_mul(qa[:], eyt[:], COEF[CHK * j])
                    nc.vector.scalar_tensor_tensor(
                        qa[:], vts[:, 0, :], COEF[CHK * j + 1], qa[:],
                        op0=OP.mult, op1=OP.add,
                    )
                    qp = chp.tile([P, B], BF16, tag="qp", bufs=4)
                    nc.vector.scalar_tensor_tensor(
                        qp[:], vts[:, 1, :], COEF[CHK * j + 2], qa[:],
                        op0=OP.mult, op1=OP.add,
                    )
                    if j == NQ - 1:
                        # R0 = Q3 straight into the Horner rhs buffer
                        nc.vector.scalar_tensor_tensor(
                            vbufB[:, t, :], vts[:, 2, :], COEF[CHK * j + 3],
                            qp[:], op0=OP.mult, op1=OP.add,
                        )
                    else:
                        qp2 = chp.tile([P, B], BF16, tag="qp", bufs=4)
                        nc.vector.scalar_tensor_tensor(
                            qp2[:], vts[:, 2, :], COEF[CHK * j + 3], qp[:],
                            op0=OP.mult, op1=OP.add,
                        )
                        nc.sync.dma_start(qd_t[j][t], qp2[:])

            for p in range(2, CHK + 1):
                rhs = bufs[p % 2]
                nxt = bufs[(p + 1) % 2] if p < CHK else None
                mid = None
                if p == 3:
                    # Q0..Q2 for tiles 0..15 (V3[t] stored by eviction t of
                    # this phase; qd/vf DRAM only -- no SBUF rhs conflict)
                    mid = {}
                    for i in range(NT // 2):
                        mid.setdefault(2 * i + 1, []).append(
                            lambda t=i: qpart_tile(t, [2, 1, 0])
                        )
                if p == CHK:
                    # Q0..Q2 for tiles 16..31, Q3 (=R0 -> vbufB) for all,
                    # plus the W4 gather chunks
                    mid = {}
                    for i in range(NT // 2):
                        mid.setdefault(i, []).append(
                            lambda t=NT // 2 + i: qpart_tile(t, [2, 1, 0])
                        )
                    for i in range(NT // 2):
                        mid.setdefault(NT // 2 + i, []).append(
                            lambda t=2 * i: qpart_tile(t, [NQ - 1])
                        )
                        mid.setdefault(NT // 2 + i, []).append(
                            lambda t=2 * i + 1: qpart_tile(t, [NQ - 1])
                        )
                    for q in range(SPL - 1):
                        mid.setdefault((q + 1) * TQ - 1, []).append(
                            gather(cc_in1[q], cc_w4[q])
                        )
                mm_phase(
                    lambda mt: adj_strips[mt],
                    rhs,
                    lambda mt, ps, p=p, nxt=nxt: evac_power(mt, ps, p, nxt),
                    mid=mid,
                )

            gather(cc_in1[SPL - 1], cc_w4[SPL - 1])()

            # -------- Horner: R = W4 @ R + Q_j, j=2..0 --------
            # j=2: rhs=B (Q3=R0) -> A ; j=1: A -> B ; j=0: B -> A (=H5)
            for j in range(NQ - 2, -1, -1):
                rhs = bufs[(j + 1) % 2]
                nxt = bufs[j % 2]

                def evac_horner(mt, ps, j=j, nxt=nxt):
                    qt = chp.tile([P, B], BF16, tag="qld", bufs=3)
                    nc.sync.dma_start(qt[:], qd_t[j][mt])
                    if j > 0:
                        nc.vector.tensor_tensor(
                            nxt[:, mt, :], ps[:], qt[:], op=OP.add
                        )
                    else:
                        rf = chp.tile([P, B], F32, tag="evf", bufs=2)
                        nc.vector.tensor_tensor(rf[:], ps[:], qt[:], op=OP.add)
                        nc.vector.tensor_copy(nxt[:, mt, :], rf[:])  # H5 bf16
                        nc.sync.dma_start(
                            cc2_t[mt // TQ][mt % TQ], nxt[:, mt, :]
                        )
                        stat_pair(rf, acc_cs5, acc_ss5)

                mid = None
                if j == 0:
                    mid = {
                        (q + 1) * TQ - 1: [gather(cc_in2[q], cc_h5[q])]
                        for q in range(SPL - 1)
                    }
                mm_phase(
                    lambda mt: [sq_[mt // NB, mt % NB] for sq_ in ccw4_s],
                    rhs,
                    evac_horner,
                    mid=mid,
                )

            gather(cc_in2[SPL - 1], cc_h5[SPL - 1])()

            # ---------------- H10 = H5 @ H5_blk + stats ----------------
            h5buf = bufs[0]

            def evac_h10(mt, ps):
                rf = chp.tile([P, B], F32, tag="evf", bufs=2)
                nc.vector.tensor_copy(rf[:], ps[:])
                stat_pair(rf, acc_cs10, acc_ss10)

            mm_phase(
                lambda mt: [sq_[mt // NB, mt % NB] for sq_ in cch5_s],
                h5buf,
                evac_h10,
            )

            # ---------------- output ----------------
            for i, acc in enumerate([acc_cs5, acc_ss5, acc_cs10, acc_ss10]):
                nc.sync.dma_start(out[i : i + 1, :], acc[:])

    nc.compile()
    return nc


_NC_CACHE = None


def _get_nc():
    global _NC_CACHE
    if _NC_CACHE is None:
        _NC_CACHE = build_nc()
    return _NC_CACHE


def _make_in_maps(pos: np.ndarray):
    x = pos.astype(np.float32)
    sq = (x * x).sum(axis=1, dtype=np.float32)
    ones = np.ones(N, dtype=np.float32)
    augL = np.stack([-2.0 * x[:, 0], -2.0 * x[:, 1], -2.0 * x[:, 2], sq, ones])
    augR = np.stack([x[:, 0], x[:, 1], x[:, 2], ones, sq])
    augL = np.ascontiguousarray(augL).astype(ml_dtypes.bfloat16)
    augR = np.ascontiguousarray(augR).astype(ml_dtypes.bfloat16)
    in_maps = []
    for c in range(C):
        eye = np.eye(N, B, k=-B * c, dtype=np.float32).astype(ml_dtypes.bfloat16)
        augRb = np.ascontiguousarray(augR[:, B * c : B * (c + 1)])
        in_maps.append(
            {"augL": augL, "augR": augR, "augRb": augRb, "eye_blk": eye}
        )
    return in_maps


def _reduce_stats(results):
    cs5 = np.concatenate([results[c]["out"][0] for c in range(C)]).astype(np.float64)
    ss5 = np.concatenate([results[c]["out"][1] for c in range(C)]).astype(np.float64)
    cs10 = np.concatenate([results[c]["out"][2] for c in range(C)]).astype(np.float64)
    ss10 = np.concatenate([results[c]["out"][3] for c in range(C)]).astype(np.float64)
    total = 0.0
    for cs, ss in ((cs5, ss5), (cs10, ss10)):
        mean = cs / N
        var = (ss - N * mean**2) / (N - 1)
        std = np.sqrt(np.maximum(var, 0.0))
        total += np.sum(std / (mean + 1e-6))
    return np.float32(total / (N * 2))


def kernel(optimized_positions: np.ndarray) -> np.ndarray:
    pos = np.ascontiguousarray(optimized_positions, dtype=np.float32)
    assert pos.shape == (N, 3)
    nc = _get_nc()
    res = run_bass_kernel_spmd(nc, _make_in_maps(pos), core_ids=list(range(C)))
    return _reduce_stats(res.results)


if __name__ == "__main__":
    rng = np.random.default_rng(0)
    pos = rng.standard_normal((N, 3)).astype(np.float32)
    print("scalar =", kernel(optimized_positions=pos))
